# revision 28
# baseline (speedup 1.0000x reference)
"""Trainium2 Bass kernel for the nms_detection problem.

Pipeline per NeuronCore (8 cores, pure data-parallel over the batch of 64):
  - stream the core's x shard [8, 2048, 14, 14] from HBM as per-sample
    [128, 3136] tiles (partition p holds channels 16p..16p+15, contiguous
    12.5 KB per partition),
  - exact-f32 channel reduction: in-place halving adds on VectorE fold
    16 channel groups down to 2, then fp32 TensorE matmuls with
    per-sample selector columns finish the reduction and accumulate
    sample s's channel-sum map into row s of one shared PSUM tile
    (other rows receive exact +0.0),
  - sliding-window average pooling for the 3 window shapes via shifted
    adds on VectorE (scale folded into the first multiply),
  - greedy NMS per window group, vectorized across the 8 samples
    (samples on partitions): max -> one-hot (is_equal) -> index via
    min-reduce of onehot*(iota-1e6) -> suppression row gathered with a
    TensorE transpose + 0/1 matmul against the IoU>=thresh table ->
    masked scores -= 1e30 * suppression.
Outputs per core: [8, 375] f32 = [7 idx | 7 scores | 361 all_scores].
"""

import sys

for _p in ("/opt/trn_rl_repo", "/root/.axon_site/_ro/trn_rl_repo"):
    if _p not in sys.path:
        sys.path.append(_p)

import numpy as np

import concourse.bass as bass
import concourse.mybir as mybir
from concourse.tile import TileContext
from concourse.bass_utils import run_bass_kernel_spmd

F32 = mybir.dt.float32
ALU = mybir.AluOpType
AXX = mybir.AxisListType.X

N_CORES = 8
B = 64
SPC = B // N_CORES          # samples per core
C = 2048
FM = 14
POS = FM * FM               # 196
CC = 16                     # channels folded into the free dim per partition
FLAT = C * POS              # 401408 per sample
PF = FLAT // 128            # 3136 free elems per partition

RATIOS = [(4, 4), (3, 5), (5, 3)]
N_LIST = [2, 3, 2]
IOU_THRESH = 0.25
WINDOW_NUMS = [(FM - rh + 1) * (FM - rw + 1) for rh, rw in RATIOS]  # [121,120,120]
OFFSETS = np.concatenate([[0], np.cumsum(WINDOW_NUMS)]).astype(int)  # [0,121,241,361]
NWIN = int(OFFSETS[-1])     # 361
NPROP = sum(N_LIST)         # 7
OUTW = NPROP * 2 + NWIN     # 375
BIGI = 1.0e6
BIGS = 1.0e30
WMAX = max(WINDOW_NUMS)     # 121


def _split_multi_waits(nc, max_keep=1):
    """walrus in this toolchain rejects instructions carrying more than one
    semaphore wait (e.g. the TileContext tail drain). Hoist extra waits onto
    EventSemaphore instructions inserted just before, on the same engine
    (engines execute in order, so semantics are unchanged)."""
    n = 0
    for f in nc.m.functions:
        for bb in f.blocks:
            out = []
            changed = False
            for ins in bb.instructions:
                si = ins.sync_info
                waits = list(si.on_wait) if (si is not None and si.on_wait) else []
                if len(waits) > max_keep:
                    changed = True
                    for w in waits[:-max_keep]:
                        n += 1
                        es = mybir.InstEventSemaphore(
                            name=f"WSPLIT-{n}", ins=[], outs=[],
                            sync_info=mybir.SyncInfo(on_wait=[w], on_update=[]))
                        es.engine = ins.engine
                        out.append(es)
                    ins.sync_info = mybir.SyncInfo(
                        on_wait=waits[-max_keep:],
                        on_update=list(si.on_update) if si.on_update else [])
                out.append(ins)
            if changed:
                bb.instructions = out
    return n


def _build_module():
    nc = bass.Bass()
    x_ext = nc.declare_dram_parameter("x", [SPC, FLAT], F32, isOutput=False)
    sel_ext = nc.declare_dram_parameter("sel", [128, SPC * SPC], F32, isOutput=False)
    id8_ext = nc.declare_dram_parameter("id8", [SPC, SPC], F32, isOutput=False)
    sm_ext = nc.declare_dram_parameter("smask", [WMAX, NWIN], F32, isOutput=False)
    io_ext = nc.declare_dram_parameter("iotamb", [SPC, NWIN], F32, isOutput=False)
    out_ext = nc.declare_dram_parameter("out", [SPC, OUTW], F32, isOutput=True)

    with TileContext(nc) as tc:
        with tc.tile_pool(name="const", bufs=1) as cpool, \
             tc.tile_pool(name="xs", bufs=4) as xpool, \
             tc.tile_pool(name="work", bufs=2) as wpool, \
             tc.tile_pool(name="pss", bufs=2, space="PSUM") as pss, \
             tc.tile_pool(name="pst", bufs=2, space="PSUM") as pstp, \
             tc.tile_pool(name="psg", bufs=2, space="PSUM") as psgp:

            sel_t = cpool.tile([128, SPC * SPC], F32)
            id8_t = cpool.tile([SPC, SPC], F32)
            sm_t = cpool.tile([WMAX, NWIN], F32)
            io_t = cpool.tile([SPC, NWIN], F32)

            s_all = cpool.tile([SPC, POS], F32)
            out_sb = cpool.tile([SPC, OUTW], F32)

            # ---- channel reduction; selector columns accumulate sample s
            # ---- into PSUM row s (other rows get exact +0.0). DVE folds
            # ---- 16->4 channel groups; PE absorbs the last 4x via two
            # ---- accumulating fp32 matmuls per sample.
            ps = pss.tile([SPC, POS], F32)
            HPF = PF // 2
            for s in range(SPC):
                xs = x_ext[s].rearrange("(p f) -> p f", p=128)
                last = s == SPC - 1
                for hh in range(2 if last else 1):
                    xt = xpool.tile([128, PF], F32)
                    if last:
                        nc.sync.dma_start(out=xt[:, 0:HPF],
                                          in_=xs[:, HPF * hh:HPF * (hh + 1)])
                        w = HPF
                    else:
                        deng = nc.gpsimd if s == 0 else nc.sync
                        deng.dma_start(out=xt[:], in_=xs[:])
                        w = PF
                    if s == 0:
                        for t, csrc in ((sel_t, sel_ext), (id8_t, id8_ext),
                                        (sm_t, sm_ext), (io_t, io_ext)):
                            nc.scalar.dma_start(out=t[:], in_=csrc[:])
                    while w > POS if last else w > 2 * POS:
                        w //= 2
                        nc.vector.tensor_add(xt[:, 0:w], xt[:, 0:w],
                                             xt[:, w:2 * w])
                    for c in range(1 if last else 2):
                        nc.tensor.matmul(ps[:],
                                         lhsT=sel_t[:, SPC * s:SPC * (s + 1)],
                                         rhs=xt[:, POS * c:POS * (c + 1)],
                                         start=(s == 0 and hh == 0 and c == 0),
                                         stop=(last and hh == 1))

            # ---- pooling: shared unscaled rowsums, scale folded into the
            # ---- column pass. s_all copied from PSUM once (DVE reads of
            # ---- two PSUM operands in one op are not allowed).
            nc.vector.tensor_copy(s_all[:], ps[:])
            sv = s_all[:].rearrange("p (i j) -> p i j", i=FM)
            r3t = wpool.tile([SPC, FM * 12], F32, tag="r3")
            r3 = r3t[:].rearrange("p (i j) -> p i j", i=FM)
            nc.vector.tensor_add(r3[:, :, :], sv[:, :, 0:12], sv[:, :, 1:13])
            nc.vector.tensor_add(r3[:, :, :], r3[:, :, :], sv[:, :, 2:14])
            r4t = wpool.tile([SPC, FM * 11], F32, tag="r4")
            r4 = r4t[:].rearrange("p (i j) -> p i j", i=FM)
            nc.vector.tensor_add(r4[:, :, :], r3[:, :, 0:11], sv[:, :, 3:14])
            r5t = wpool.tile([SPC, FM * 10], F32, tag="r5")
            r5 = r5t[:].rearrange("p (i j) -> p i j", i=FM)
            nc.vector.tensor_add(r5[:, :, :], r4[:, :, 0:10], sv[:, :, 4:14])
            rows = {4: r4, 5: r5, 3: r3}
            for g, (rh, rw) in enumerate(RATIOS):
                st, en = int(OFFSETS[g]), int(OFFSETS[g + 1])
                jw, ih = FM - rw + 1, FM - rh + 1
                scale = 1.0 / (rh * rw)
                rp = rows[rw]
                dst = out_sb[:, 2 * NPROP + st:2 * NPROP + en].rearrange(
                    "p (i j) -> p i j", i=ih)
                nc.vector.tensor_scalar(dst[:, :, :], rp[:, 0:ih, :],
                                        scale, None, ALU.mult)
                for hd in range(1, rh):
                    nc.vector.scalar_tensor_tensor(
                        out=dst[:, :, :], in0=rp[:, hd:hd + ih, :], scalar=scale,
                        in1=dst[:, :, :], op0=ALU.mult, op1=ALU.add)

            # ---- greedy NMS per group, samples vectorized on partitions.
            # Index extraction: scr = (masked == max) * iota2 where
            # iota2[j] = (NWIN - j)/512 > 0; reduce_max(scr) encodes the
            # first-argmax index (host decodes idx = NWIN - 512*v).
            # Suppression row gather: scrT.T @ smask = v * S[idx, :] with
            # v > 0, so masked += -1e30 * supp kills suppressed windows.
            col = 0
            for g, (n_g, wg) in enumerate(zip(N_LIST, WINDOW_NUMS)):
                st, en = int(OFFSETS[g]), int(OFFSETS[g + 1])
                masked = wpool.tile([SPC, wg], F32, tag=f"masked{g}")
                for k in range(n_g):
                    src_ap = (out_sb[:, 2 * NPROP + st:2 * NPROP + en]
                              if k == 0 else masked[:])
                    sc_col = out_sb[:, NPROP + col:NPROP + col + 1]
                    nc.vector.tensor_reduce(sc_col, src_ap, axis=AXX, op=ALU.max)
                    scr = wpool.tile([SPC, wg], F32, tag=f"scr{g}")
                    nc.vector.scalar_tensor_tensor(
                        out=scr[:], in0=src_ap, scalar=sc_col,
                        in1=io_t[:, st:en], op0=ALU.is_equal, op1=ALU.mult,
                        accum_out=out_sb[:, col:col + 1])
                    if k < n_g - 1:
                        pT = pstp.tile([WMAX, SPC], F32, tag="pT")
                        nc.tensor.transpose(pT[0:wg, :], scr[:], id8_t[:])
                        ohT = wpool.tile([WMAX, SPC], F32, tag=f"ohT{g}")
                        nc.vector.tensor_copy(ohT[0:wg, :], pT[0:wg, :])
                        pS = psgp.tile([SPC, WMAX], F32, tag="pS")
                        nc.tensor.matmul(pS[:, 0:wg], lhsT=ohT[0:wg, :],
                                         rhs=sm_t[0:wg, st:en],
                                         start=True, stop=True)
                        nc.vector.scalar_tensor_tensor(
                            out=masked[:], in0=pS[:, 0:wg], scalar=-BIGS,
                            in1=src_ap, op0=ALU.mult, op1=ALU.add)
                    col += 1

            nc.sync.dma_start(out=out_ext[:, 2 * NPROP:],
                              in_=out_sb[:, 2 * NPROP:])
            nc.sync.dma_start(out=out_ext[:, 0:2 * NPROP],
                              in_=out_sb[:, 0:2 * NPROP])

    _split_multi_waits(nc)
    return nc


def _host_constants(coordinates):
    co = np.asarray(coordinates).astype(np.int64)  # [361, 4]
    sel = np.zeros((128, SPC * SPC), np.float32)
    for s in range(SPC):
        sel[:, SPC * s + s] = 1.0
    id8 = np.eye(SPC, dtype=np.float32)
    smask = np.zeros((WMAX, NWIN), np.float32)
    for g in range(len(RATIOS)):
        st, en = int(OFFSETS[g]), int(OFFSETS[g + 1])
        b = co[st:en]
        x0 = np.maximum(b[:, None, 0], b[None, :, 0])
        y0 = np.maximum(b[:, None, 1], b[None, :, 1])
        x1 = np.minimum(b[:, None, 2], b[None, :, 2])
        y1 = np.minimum(b[:, None, 3], b[None, :, 3])
        inter = np.maximum(x1 - x0, 0) * np.maximum(y1 - y0, 0)
        area = (b[:, 2] - b[:, 0]) * (b[:, 3] - b[:, 1])
        union = area[:, None] + area[None, :] - inter
        # iou >= 0.25 exactly, in integer arithmetic
        sup = (4 * inter >= union).astype(np.float32)
        smask[0:en - st, st:en] = sup
    iotamb = ((NWIN - np.arange(NWIN, dtype=np.float64)) / 512.0).astype(np.float32)
    iotamb = np.broadcast_to(iotamb, (SPC, NWIN)).copy()
    return sel, id8, smask, iotamb


_NC_CACHE = {}


def _get_module():
    if "nc" not in _NC_CACHE:
        _NC_CACHE["nc"] = _build_module()
    return _NC_CACHE["nc"]


def run(inputs, trace=False):
    x = np.asarray(inputs["x"], dtype=np.float32)
    coordinates = np.asarray(inputs["coordinates"])
    assert x.shape == (B, C, FM, FM), x.shape
    sel, id8, smask, iotamb = _host_constants(coordinates)

    nc = _get_module()
    in_maps = []
    for i in range(N_CORES):
        shard = np.ascontiguousarray(
            x[i * SPC:(i + 1) * SPC].reshape(SPC, FLAT))
        in_maps.append({
            "x": shard, "sel": sel, "id8": id8,
            "smask": smask, "iotamb": iotamb,
        })
    res = run_bass_kernel_spmd(nc, in_maps, core_ids=list(range(N_CORES)),
                               trace=trace)
    outs = [res.results[i]["out"] for i in range(N_CORES)]
    full = np.concatenate(outs, axis=0)  # [64, 375]
    idx = np.rint(NWIN - 512.0 * full[:, 0:NPROP].astype(np.float64)).astype(np.int32)
    prop_scores = full[:, NPROP:2 * NPROP].astype(np.float32)
    all_scores = full[:, 2 * NPROP:].astype(np.float32)
    return (idx, prop_scores, all_scores), res


def kernel(**inputs):
    (idx, prop_scores, all_scores), _ = run(inputs, trace=False)
    return idx, prop_scores, all_scores


# revision 34
# speedup vs baseline: 1.2116x; 1.2116x over previous
"""Trainium2 Bass kernel for the nms_detection problem.

Pipeline per NeuronCore (8 cores, pure data-parallel over the batch of 64):
  - stream the core's x shard [8, 2048, 14, 14] from HBM as per-sample
    [128, 3136] tiles (partition p holds channels 16p..16p+15, contiguous
    12.5 KB per partition),
  - exact-f32 channel reduction: in-place halving adds on VectorE fold
    16 channel groups down to 2, then fp32 TensorE matmuls with
    per-sample selector columns finish the reduction and accumulate
    sample s's channel-sum map into row s of one shared PSUM tile
    (other rows receive exact +0.0),
  - sliding-window average pooling for the 3 window shapes via shifted
    adds on VectorE (scale folded into the first multiply),
  - greedy NMS per window group, vectorized across the 8 samples
    (samples on partitions): max -> one-hot (is_equal) -> index via
    min-reduce of onehot*(iota-1e6) -> suppression row gathered with a
    TensorE transpose + 0/1 matmul against the IoU>=thresh table ->
    masked scores -= 1e30 * suppression.
Outputs per core: [8, 375] f32 = [7 idx | 7 scores | 361 all_scores].
"""

import sys

for _p in ("/opt/trn_rl_repo", "/root/.axon_site/_ro/trn_rl_repo"):
    if _p not in sys.path:
        sys.path.append(_p)

import numpy as np

import concourse.bass as bass
import concourse.mybir as mybir
from concourse.tile import TileContext
from concourse.bass_utils import run_bass_kernel_spmd

F32 = mybir.dt.float32
ALU = mybir.AluOpType
AXX = mybir.AxisListType.X

N_CORES = 8
B = 64
SPC = B // N_CORES          # samples per core
C = 2048
FM = 14
POS = FM * FM               # 196
CC = 16                     # channels folded into the free dim per partition
FLAT = C * POS              # 401408 per sample
PF = FLAT // 128            # 3136 free elems per partition

RATIOS = [(4, 4), (3, 5), (5, 3)]
N_LIST = [2, 3, 2]
IOU_THRESH = 0.25
WINDOW_NUMS = [(FM - rh + 1) * (FM - rw + 1) for rh, rw in RATIOS]  # [121,120,120]
OFFSETS = np.concatenate([[0], np.cumsum(WINDOW_NUMS)]).astype(int)  # [0,121,241,361]
NWIN = int(OFFSETS[-1])     # 361
NPROP = sum(N_LIST)         # 7
OUTW = NPROP * 2 + NWIN     # 375
BIGI = 1.0e6
BIGS = 1.0e30
WMAX = max(WINDOW_NUMS)     # 121


def _split_multi_waits(nc, max_keep=1):
    """walrus in this toolchain rejects instructions carrying more than one
    semaphore wait (e.g. the TileContext tail drain). Hoist extra waits onto
    EventSemaphore instructions inserted just before, on the same engine
    (engines execute in order, so semantics are unchanged)."""
    n = 0
    for f in nc.m.functions:
        for bb in f.blocks:
            out = []
            changed = False
            for ins in bb.instructions:
                si = ins.sync_info
                waits = list(si.on_wait) if (si is not None and si.on_wait) else []
                if len(waits) > max_keep:
                    changed = True
                    for w in waits[:-max_keep]:
                        n += 1
                        es = mybir.InstEventSemaphore(
                            name=f"WSPLIT-{n}", ins=[], outs=[],
                            sync_info=mybir.SyncInfo(on_wait=[w], on_update=[]))
                        es.engine = ins.engine
                        out.append(es)
                    ins.sync_info = mybir.SyncInfo(
                        on_wait=waits[-max_keep:],
                        on_update=list(si.on_update) if si.on_update else [])
                out.append(ins)
            if changed:
                bb.instructions = out
    return n


def _build_module():
    nc = bass.Bass()
    x_ext = nc.declare_dram_parameter("x", [SPC, FLAT], F32, isOutput=False)
    sel_ext = nc.declare_dram_parameter("sel", [128, SPC * SPC], F32, isOutput=False)
    id8_ext = nc.declare_dram_parameter("id8", [SPC, SPC], F32, isOutput=False)
    sm_ext = nc.declare_dram_parameter("smask", [WMAX, NWIN], F32, isOutput=False)
    io_ext = nc.declare_dram_parameter("iotamb", [SPC, NWIN], F32, isOutput=False)
    out_ext = nc.declare_dram_parameter("out", [SPC, OUTW], F32, isOutput=True)

    with TileContext(nc) as tc:
        with tc.tile_pool(name="const", bufs=1) as cpool, \
             tc.tile_pool(name="xs", bufs=4) as xpool, \
             tc.tile_pool(name="work", bufs=2) as wpool, \
             tc.tile_pool(name="pss", bufs=2, space="PSUM") as pss, \
             tc.tile_pool(name="pst", bufs=2, space="PSUM") as pstp, \
             tc.tile_pool(name="psg", bufs=2, space="PSUM") as psgp:

            sel_t = cpool.tile([128, SPC * SPC], F32)
            id8_t = cpool.tile([SPC, SPC], F32)
            sm_t = cpool.tile([WMAX, NWIN], F32)
            io_t = cpool.tile([SPC, NWIN], F32)

            s_all = cpool.tile([SPC, POS], F32)
            out_sb = cpool.tile([SPC, OUTW], F32)

            # ---- channel reduction; selector columns accumulate sample s
            # ---- into PSUM row s (other rows get exact +0.0). DVE folds
            # ---- 16->4 channel groups; PE absorbs the last 4x via two
            # ---- accumulating fp32 matmuls per sample.
            ps = pss.tile([SPC, POS], F32)
            HPF = PF // 2
            for s in range(SPC):
                xs = x_ext[s].rearrange("(p f) -> p f", p=128)
                last = s == SPC - 1
                for hh in range(2 if last else 1):
                    xt = xpool.tile([128, PF], F32)
                    if last:
                        nc.sync.dma_start(out=xt[:, 0:HPF],
                                          in_=xs[:, HPF * hh:HPF * (hh + 1)])
                        w = HPF
                    else:
                        nc.sync.dma_start(out=xt[:], in_=xs[:])
                        w = PF
                    if s == 0:
                        for t, csrc in ((sel_t, sel_ext), (id8_t, id8_ext),
                                        (sm_t, sm_ext), (io_t, io_ext)):
                            nc.scalar.dma_start(out=t[:], in_=csrc[:])
                    while w > POS if last else w > 2 * POS:
                        w //= 2
                        nc.vector.tensor_add(xt[:, 0:w], xt[:, 0:w],
                                             xt[:, w:2 * w])
                    for c in range(1 if last else 2):
                        nc.tensor.matmul(ps[:],
                                         lhsT=sel_t[:, SPC * s:SPC * (s + 1)],
                                         rhs=xt[:, POS * c:POS * (c + 1)],
                                         start=(s == 0 and hh == 0 and c == 0),
                                         stop=(last and hh == 1))

            # ---- pooling: shared unscaled rowsums, scale folded into the
            # ---- column pass. s_all copied from PSUM once (DVE reads of
            # ---- two PSUM operands in one op are not allowed).
            nc.vector.tensor_copy(s_all[:], ps[:])
            sv = s_all[:].rearrange("p (i j) -> p i j", i=FM)
            r3t = wpool.tile([SPC, FM * 12], F32, tag="r3")
            r3 = r3t[:].rearrange("p (i j) -> p i j", i=FM)
            nc.vector.tensor_add(r3[:, :, :], sv[:, :, 0:12], sv[:, :, 1:13])
            nc.vector.tensor_add(r3[:, :, :], r3[:, :, :], sv[:, :, 2:14])
            r4t = wpool.tile([SPC, FM * 11], F32, tag="r4")
            r4 = r4t[:].rearrange("p (i j) -> p i j", i=FM)
            nc.vector.tensor_add(r4[:, :, :], r3[:, :, 0:11], sv[:, :, 3:14])
            r5t = wpool.tile([SPC, FM * 10], F32, tag="r5")
            r5 = r5t[:].rearrange("p (i j) -> p i j", i=FM)
            nc.vector.tensor_add(r5[:, :, :], r4[:, :, 0:10], sv[:, :, 4:14])
            rows = {4: r4, 5: r5, 3: r3}
            for g, (rh, rw) in enumerate(RATIOS):
                st, en = int(OFFSETS[g]), int(OFFSETS[g + 1])
                jw, ih = FM - rw + 1, FM - rh + 1
                scale = 1.0 / (rh * rw)
                rp = rows[rw]
                dst = out_sb[:, 2 * NPROP + st:2 * NPROP + en].rearrange(
                    "p (i j) -> p i j", i=ih)
                nc.vector.tensor_scalar(dst[:, :, :], rp[:, 0:ih, :],
                                        scale, None, ALU.mult)
                for hd in range(1, rh):
                    nc.vector.scalar_tensor_tensor(
                        out=dst[:, :, :], in0=rp[:, hd:hd + ih, :], scalar=scale,
                        in1=dst[:, :, :], op0=ALU.mult, op1=ALU.add)

            # ---- greedy NMS per group, samples vectorized on partitions.
            # Index extraction: scr = (masked == max) * iota2 where
            # iota2[j] = (NWIN - j)/512 > 0; reduce_max(scr) encodes the
            # first-argmax index (host decodes idx = NWIN - 512*v).
            # Suppression row gather: scrT.T @ smask = v * S[idx, :] with
            # v > 0, so masked += -1e30 * supp kills suppressed windows.
            col = 0
            for g, (n_g, wg) in enumerate(zip(N_LIST, WINDOW_NUMS)):
                st, en = int(OFFSETS[g]), int(OFFSETS[g + 1])
                masked = wpool.tile([SPC, wg], F32, tag=f"masked{g}")
                for k in range(n_g):
                    src_ap = (out_sb[:, 2 * NPROP + st:2 * NPROP + en]
                              if k == 0 else masked[:])
                    sc_col = out_sb[:, NPROP + col:NPROP + col + 1]
                    nc.vector.tensor_reduce(sc_col, src_ap, axis=AXX, op=ALU.max)
                    scr = wpool.tile([SPC, wg], F32, tag=f"scr{g}")
                    nc.vector.scalar_tensor_tensor(
                        out=scr[:], in0=src_ap, scalar=sc_col,
                        in1=io_t[:, st:en], op0=ALU.is_equal, op1=ALU.mult,
                        accum_out=out_sb[:, col:col + 1])
                    if k < n_g - 1:
                        pT = pstp.tile([WMAX, SPC], F32, tag="pT")
                        nc.tensor.transpose(pT[0:wg, :], scr[:], id8_t[:])
                        ohT = wpool.tile([WMAX, SPC], F32, tag=f"ohT{g}")
                        nc.vector.tensor_copy(ohT[0:wg, :], pT[0:wg, :])
                        pS = psgp.tile([SPC, WMAX], F32, tag="pS")
                        nc.tensor.matmul(pS[:, 0:wg], lhsT=ohT[0:wg, :],
                                         rhs=sm_t[0:wg, st:en],
                                         start=True, stop=True)
                        nc.vector.scalar_tensor_tensor(
                            out=masked[:], in0=pS[:, 0:wg], scalar=-BIGS,
                            in1=src_ap, op0=ALU.mult, op1=ALU.add)
                    col += 1

            nc.sync.dma_start(out=out_ext[:, 2 * NPROP:],
                              in_=out_sb[:, 2 * NPROP:])
            nc.sync.dma_start(out=out_ext[:, 0:2 * NPROP],
                              in_=out_sb[:, 0:2 * NPROP])

    _split_multi_waits(nc)
    return nc


def _host_constants(coordinates):
    co = np.asarray(coordinates).astype(np.int64)  # [361, 4]
    sel = np.zeros((128, SPC * SPC), np.float32)
    for s in range(SPC):
        sel[:, SPC * s + s] = 1.0
    id8 = np.eye(SPC, dtype=np.float32)
    smask = np.zeros((WMAX, NWIN), np.float32)
    for g in range(len(RATIOS)):
        st, en = int(OFFSETS[g]), int(OFFSETS[g + 1])
        b = co[st:en]
        x0 = np.maximum(b[:, None, 0], b[None, :, 0])
        y0 = np.maximum(b[:, None, 1], b[None, :, 1])
        x1 = np.minimum(b[:, None, 2], b[None, :, 2])
        y1 = np.minimum(b[:, None, 3], b[None, :, 3])
        inter = np.maximum(x1 - x0, 0) * np.maximum(y1 - y0, 0)
        area = (b[:, 2] - b[:, 0]) * (b[:, 3] - b[:, 1])
        union = area[:, None] + area[None, :] - inter
        # iou >= 0.25 exactly, in integer arithmetic
        sup = (4 * inter >= union).astype(np.float32)
        smask[0:en - st, st:en] = sup
    iotamb = ((NWIN - np.arange(NWIN, dtype=np.float64)) / 512.0).astype(np.float32)
    iotamb = np.broadcast_to(iotamb, (SPC, NWIN)).copy()
    return sel, id8, smask, iotamb


_NC_CACHE = {}


def _get_module():
    if "nc" not in _NC_CACHE:
        _NC_CACHE["nc"] = _build_module()
    return _NC_CACHE["nc"]


def run(inputs, trace=False):
    x = np.asarray(inputs["x"], dtype=np.float32)
    coordinates = np.asarray(inputs["coordinates"])
    assert x.shape == (B, C, FM, FM), x.shape
    sel, id8, smask, iotamb = _host_constants(coordinates)

    nc = _get_module()
    in_maps = []
    for i in range(N_CORES):
        shard = np.ascontiguousarray(
            x[i * SPC:(i + 1) * SPC].reshape(SPC, FLAT))
        in_maps.append({
            "x": shard, "sel": sel, "id8": id8,
            "smask": smask, "iotamb": iotamb,
        })
    res = run_bass_kernel_spmd(nc, in_maps, core_ids=list(range(N_CORES)),
                               trace=trace)
    outs = [res.results[i]["out"] for i in range(N_CORES)]
    full = np.concatenate(outs, axis=0)  # [64, 375]
    idx = np.rint(NWIN - 512.0 * full[:, 0:NPROP].astype(np.float64)).astype(np.int32)
    prop_scores = full[:, NPROP:2 * NPROP].astype(np.float32)
    all_scores = full[:, 2 * NPROP:].astype(np.float32)
    return (idx, prop_scores, all_scores), res


def kernel(**inputs):
    (idx, prop_scores, all_scores), _ = run(inputs, trace=False)
    return idx, prop_scores, all_scores


# revision 35
# speedup vs baseline: 1.2227x; 1.0092x over previous
"""Trainium2 Bass kernel for the nms_detection problem.

Pipeline per NeuronCore (8 cores, pure data-parallel over the batch of 64):
  - stream the core's x shard [8, 2048, 14, 14] from HBM as per-sample
    [128, 3136] tiles (partition p holds channels 16p..16p+15, contiguous
    12.5 KB per partition),
  - exact-f32 channel reduction: in-place halving adds on VectorE fold
    16 channel groups down to 2, then fp32 TensorE matmuls with
    per-sample selector columns finish the reduction and accumulate
    sample s's channel-sum map into row s of one shared PSUM tile
    (other rows receive exact +0.0); the last sample streams as two
    halves so its post-stream serial chain is shorter,
  - sliding-window average pooling for the 3 window shapes via shifted
    adds on VectorE (scale folded into the first multiply),
  - greedy NMS per window group, vectorized across the 8 samples
    (samples on partitions): max -> one-hot (is_equal) -> index via
    min-reduce of onehot*(iota-1e6) -> suppression row gathered with a
    TensorE transpose + 0/1 matmul against the IoU>=thresh table ->
    masked scores -= 1e30 * suppression.
Outputs per core: [8, 375] f32 = [7 idx | 7 scores | 361 all_scores].
"""

import sys

for _p in ("/opt/trn_rl_repo", "/root/.axon_site/_ro/trn_rl_repo"):
    if _p not in sys.path:
        sys.path.append(_p)

import numpy as np

import concourse.bass as bass
import concourse.mybir as mybir
from concourse.tile import TileContext
from concourse.bass_utils import run_bass_kernel_spmd

F32 = mybir.dt.float32
ALU = mybir.AluOpType
AXX = mybir.AxisListType.X

N_CORES = 8
B = 64
SPC = B // N_CORES          # samples per core
C = 2048
FM = 14
POS = FM * FM               # 196
CC = 16                     # channels folded into the free dim per partition
FLAT = C * POS              # 401408 per sample
PF = FLAT // 128            # 3136 free elems per partition

RATIOS = [(4, 4), (3, 5), (5, 3)]
N_LIST = [2, 3, 2]
IOU_THRESH = 0.25
WINDOW_NUMS = [(FM - rh + 1) * (FM - rw + 1) for rh, rw in RATIOS]  # [121,120,120]
OFFSETS = np.concatenate([[0], np.cumsum(WINDOW_NUMS)]).astype(int)  # [0,121,241,361]
NWIN = int(OFFSETS[-1])     # 361
NPROP = sum(N_LIST)         # 7
OUTW = NPROP * 2 + NWIN     # 375
BIGI = 1.0e6
BIGS = 1.0e30
WMAX = max(WINDOW_NUMS)     # 121


def _split_multi_waits(nc, max_keep=1):
    """walrus in this toolchain rejects instructions carrying more than one
    semaphore wait (e.g. the TileContext tail drain). Hoist extra waits onto
    EventSemaphore instructions inserted just before, on the same engine
    (engines execute in order, so semantics are unchanged)."""
    n = 0
    for f in nc.m.functions:
        for bb in f.blocks:
            out = []
            changed = False
            for ins in bb.instructions:
                si = ins.sync_info
                waits = list(si.on_wait) if (si is not None and si.on_wait) else []
                if len(waits) > max_keep:
                    changed = True
                    for w in waits[:-max_keep]:
                        n += 1
                        es = mybir.InstEventSemaphore(
                            name=f"WSPLIT-{n}", ins=[], outs=[],
                            sync_info=mybir.SyncInfo(on_wait=[w], on_update=[]))
                        es.engine = ins.engine
                        out.append(es)
                    ins.sync_info = mybir.SyncInfo(
                        on_wait=waits[-max_keep:],
                        on_update=list(si.on_update) if si.on_update else [])
                out.append(ins)
            if changed:
                bb.instructions = out
    return n


def _build_module():
    nc = bass.Bass()
    x_ext = nc.declare_dram_parameter("x", [SPC, FLAT], F32, isOutput=False)
    sel_ext = nc.declare_dram_parameter("sel", [128, SPC * SPC], F32, isOutput=False)
    id8_ext = nc.declare_dram_parameter("id8", [SPC, SPC], F32, isOutput=False)
    sm_ext = nc.declare_dram_parameter("smask", [WMAX, NWIN], F32, isOutput=False)
    io_ext = nc.declare_dram_parameter("iotamb", [SPC, NWIN], F32, isOutput=False)
    out_ext = nc.declare_dram_parameter("out", [SPC, OUTW], F32, isOutput=True)

    with TileContext(nc) as tc:
        with tc.tile_pool(name="const", bufs=1) as cpool, \
             tc.tile_pool(name="xs", bufs=4) as xpool, \
             tc.tile_pool(name="work", bufs=2) as wpool, \
             tc.tile_pool(name="pss", bufs=2, space="PSUM") as pss, \
             tc.tile_pool(name="pst", bufs=2, space="PSUM") as pstp, \
             tc.tile_pool(name="psg", bufs=2, space="PSUM") as psgp:

            sel_t = cpool.tile([128, SPC * SPC], F32)
            id8_t = cpool.tile([SPC, SPC], F32)
            sm_t = cpool.tile([WMAX, NWIN], F32)
            io_t = cpool.tile([SPC, NWIN], F32)

            s_all = cpool.tile([SPC, POS], F32)
            out_sb = cpool.tile([SPC, OUTW], F32)

            # ---- channel reduction; selector columns accumulate sample s
            # ---- into PSUM row s (other rows get exact +0.0). DVE folds
            # ---- 16->4 channel groups; PE absorbs the last 4x via two
            # ---- accumulating fp32 matmuls per sample.
            ps = pss.tile([SPC, POS], F32)
            HPF = PF // 2
            for s in range(SPC):
                xs = x_ext[s].rearrange("(p f) -> p f", p=128)
                last = s == SPC - 1
                for hh in range(2 if last else 1):
                    xt = xpool.tile([128, PF], F32)
                    if last:
                        nc.sync.dma_start(out=xt[:, 0:HPF],
                                          in_=xs[:, HPF * hh:HPF * (hh + 1)])
                        w = HPF
                    else:
                        nc.sync.dma_start(out=xt[:], in_=xs[:])
                        w = PF
                    if s == 0:
                        for t, csrc in ((sel_t, sel_ext), (id8_t, id8_ext),
                                        (sm_t, sm_ext), (io_t, io_ext)):
                            nc.scalar.dma_start(out=t[:], in_=csrc[:])
                    while w > POS if last else w > 2 * POS:
                        w //= 2
                        nc.vector.tensor_add(xt[:, 0:w], xt[:, 0:w],
                                             xt[:, w:2 * w])
                    for c in range(1 if last else 2):
                        nc.tensor.matmul(ps[:],
                                         lhsT=sel_t[:, SPC * s:SPC * (s + 1)],
                                         rhs=xt[:, POS * c:POS * (c + 1)],
                                         start=(s == 0 and hh == 0 and c == 0),
                                         stop=(last and hh == 1))

            # ---- pooling: shared unscaled rowsums, scale folded into the
            # ---- column pass. s_all copied from PSUM once (DVE reads of
            # ---- two PSUM operands in one op are not allowed).
            nc.vector.tensor_copy(s_all[:], ps[:])
            sv = s_all[:].rearrange("p (i j) -> p i j", i=FM)
            r3t = wpool.tile([SPC, FM * 12], F32, tag="r3")
            r3 = r3t[:].rearrange("p (i j) -> p i j", i=FM)
            nc.vector.tensor_add(r3[:, :, :], sv[:, :, 0:12], sv[:, :, 1:13])
            nc.vector.tensor_add(r3[:, :, :], r3[:, :, :], sv[:, :, 2:14])
            r4t = wpool.tile([SPC, FM * 11], F32, tag="r4")
            r4 = r4t[:].rearrange("p (i j) -> p i j", i=FM)
            nc.vector.tensor_add(r4[:, :, :], r3[:, :, 0:11], sv[:, :, 3:14])
            r5t = wpool.tile([SPC, FM * 10], F32, tag="r5")
            r5 = r5t[:].rearrange("p (i j) -> p i j", i=FM)
            nc.vector.tensor_add(r5[:, :, :], r4[:, :, 0:10], sv[:, :, 4:14])
            rows = {4: r4, 5: r5, 3: r3}
            for g, (rh, rw) in enumerate(RATIOS):
                st, en = int(OFFSETS[g]), int(OFFSETS[g + 1])
                jw, ih = FM - rw + 1, FM - rh + 1
                scale = 1.0 / (rh * rw)
                rp = rows[rw]
                dst = out_sb[:, 2 * NPROP + st:2 * NPROP + en].rearrange(
                    "p (i j) -> p i j", i=ih)
                nc.vector.tensor_scalar(dst[:, :, :], rp[:, 0:ih, :],
                                        scale, None, ALU.mult)
                for hd in range(1, rh):
                    nc.vector.scalar_tensor_tensor(
                        out=dst[:, :, :], in0=rp[:, hd:hd + ih, :], scalar=scale,
                        in1=dst[:, :, :], op0=ALU.mult, op1=ALU.add)

            # ---- greedy NMS per group, samples vectorized on partitions.
            # Index extraction: scr = (masked == max) * iota2 where
            # iota2[j] = (NWIN - j)/512 > 0; reduce_max(scr) encodes the
            # first-argmax index (host decodes idx = NWIN - 512*v).
            # Suppression row gather: scrT.T @ smask = v * S[idx, :] with
            # v > 0, so masked += -1e30 * supp kills suppressed windows.
            col = 0
            for g, (n_g, wg) in enumerate(zip(N_LIST, WINDOW_NUMS)):
                st, en = int(OFFSETS[g]), int(OFFSETS[g + 1])
                masked = wpool.tile([SPC, wg], F32, tag=f"masked{g}")
                for k in range(n_g):
                    src_ap = (out_sb[:, 2 * NPROP + st:2 * NPROP + en]
                              if k == 0 else masked[:])
                    sc_col = out_sb[:, NPROP + col:NPROP + col + 1]
                    nc.vector.tensor_reduce(sc_col, src_ap, axis=AXX, op=ALU.max)
                    scr = wpool.tile([SPC, wg], F32, tag=f"scr{g}")
                    nc.vector.scalar_tensor_tensor(
                        out=scr[:], in0=src_ap, scalar=sc_col,
                        in1=io_t[:, st:en], op0=ALU.is_equal, op1=ALU.mult,
                        accum_out=out_sb[:, col:col + 1])
                    if k < n_g - 1:
                        pT = pstp.tile([WMAX, SPC], F32, tag="pT")
                        nc.tensor.transpose(pT[0:wg, :], scr[:], id8_t[:])
                        ohT = wpool.tile([WMAX, SPC], F32, tag=f"ohT{g}")
                        nc.vector.tensor_copy(ohT[0:wg, :], pT[0:wg, :])
                        pS = psgp.tile([SPC, WMAX], F32, tag="pS")
                        nc.tensor.matmul(pS[:, 0:wg], lhsT=ohT[0:wg, :],
                                         rhs=sm_t[0:wg, st:en],
                                         start=True, stop=True)
                        nc.vector.scalar_tensor_tensor(
                            out=masked[:], in0=pS[:, 0:wg], scalar=-BIGS,
                            in1=src_ap, op0=ALU.mult, op1=ALU.add)
                    col += 1

            nc.sync.dma_start(out=out_ext[:, 2 * NPROP:],
                              in_=out_sb[:, 2 * NPROP:])
            nc.sync.dma_start(out=out_ext[:, 0:2 * NPROP],
                              in_=out_sb[:, 0:2 * NPROP])

    _split_multi_waits(nc)
    return nc


def _host_constants(coordinates):
    co = np.asarray(coordinates).astype(np.int64)  # [361, 4]
    sel = np.zeros((128, SPC * SPC), np.float32)
    for s in range(SPC):
        sel[:, SPC * s + s] = 1.0
    id8 = np.eye(SPC, dtype=np.float32)
    smask = np.zeros((WMAX, NWIN), np.float32)
    for g in range(len(RATIOS)):
        st, en = int(OFFSETS[g]), int(OFFSETS[g + 1])
        b = co[st:en]
        x0 = np.maximum(b[:, None, 0], b[None, :, 0])
        y0 = np.maximum(b[:, None, 1], b[None, :, 1])
        x1 = np.minimum(b[:, None, 2], b[None, :, 2])
        y1 = np.minimum(b[:, None, 3], b[None, :, 3])
        inter = np.maximum(x1 - x0, 0) * np.maximum(y1 - y0, 0)
        area = (b[:, 2] - b[:, 0]) * (b[:, 3] - b[:, 1])
        union = area[:, None] + area[None, :] - inter
        # iou >= 0.25 exactly, in integer arithmetic
        sup = (4 * inter >= union).astype(np.float32)
        smask[0:en - st, st:en] = sup
    iotamb = ((NWIN - np.arange(NWIN, dtype=np.float64)) / 512.0).astype(np.float32)
    iotamb = np.broadcast_to(iotamb, (SPC, NWIN)).copy()
    return sel, id8, smask, iotamb


_NC_CACHE = {}


def _get_module():
    if "nc" not in _NC_CACHE:
        _NC_CACHE["nc"] = _build_module()
    return _NC_CACHE["nc"]


def run(inputs, trace=False):
    x = np.asarray(inputs["x"], dtype=np.float32)
    coordinates = np.asarray(inputs["coordinates"])
    assert x.shape == (B, C, FM, FM), x.shape
    sel, id8, smask, iotamb = _host_constants(coordinates)

    nc = _get_module()
    in_maps = []
    for i in range(N_CORES):
        shard = np.ascontiguousarray(
            x[i * SPC:(i + 1) * SPC].reshape(SPC, FLAT))
        in_maps.append({
            "x": shard, "sel": sel, "id8": id8,
            "smask": smask, "iotamb": iotamb,
        })
    res = run_bass_kernel_spmd(nc, in_maps, core_ids=list(range(N_CORES)),
                               trace=trace)
    outs = [res.results[i]["out"] for i in range(N_CORES)]
    full = np.concatenate(outs, axis=0)  # [64, 375]
    idx = np.rint(NWIN - 512.0 * full[:, 0:NPROP].astype(np.float64)).astype(np.int32)
    prop_scores = full[:, NPROP:2 * NPROP].astype(np.float32)
    all_scores = full[:, 2 * NPROP:].astype(np.float32)
    return (idx, prop_scores, all_scores), res


def kernel(**inputs):
    (idx, prop_scores, all_scores), _ = run(inputs, trace=False)
    return idx, prop_scores, all_scores


# revision 40
# speedup vs baseline: 1.3195x; 1.0792x over previous
"""Trainium2 Bass kernel for the nms_detection problem.

Pipeline per NeuronCore (8 cores, pure data-parallel over the batch of 64):
  - stream the core's x shard [8, 2048, 14, 14] from HBM as per-sample
    [128, 3136] tiles (partition p holds channels 16p..16p+15, contiguous
    12.5 KB per partition),
  - exact-f32 channel reduction: in-place halving adds on VectorE fold
    16 channel groups down to 2, then fp32 TensorE matmuls with
    per-sample selector columns finish the reduction and accumulate
    sample s's channel-sum map into row s of one shared PSUM tile
    (other rows receive exact +0.0); the last sample streams as two
    halves so its post-stream serial chain is shorter,
  - sliding-window average pooling for the 3 window shapes via shifted
    adds on VectorE (scale folded into the first multiply),
  - greedy NMS per window group, vectorized across the 8 samples
    (samples on partitions): max -> one-hot (is_equal) -> index via
    min-reduce of onehot*(iota-1e6) -> suppression row gathered with a
    TensorE transpose + 0/1 matmul against the IoU>=thresh table ->
    masked scores -= 1e30 * suppression.
Outputs per core: [8, 375] f32 = [7 idx | 7 scores | 361 all_scores].
"""

import sys

for _p in ("/opt/trn_rl_repo", "/root/.axon_site/_ro/trn_rl_repo"):
    if _p not in sys.path:
        sys.path.append(_p)

import numpy as np

import concourse.bass as bass
import concourse.mybir as mybir
from concourse.tile import TileContext
from concourse.bass_utils import run_bass_kernel_spmd

F32 = mybir.dt.float32
ALU = mybir.AluOpType
AXX = mybir.AxisListType.X

N_CORES = 8
B = 64
SPC = B // N_CORES          # samples per core
C = 2048
FM = 14
POS = FM * FM               # 196
CC = 16                     # channels folded into the free dim per partition
FLAT = C * POS              # 401408 per sample
PF = FLAT // 128            # 3136 free elems per partition

RATIOS = [(4, 4), (3, 5), (5, 3)]
N_LIST = [2, 3, 2]
IOU_THRESH = 0.25
WINDOW_NUMS = [(FM - rh + 1) * (FM - rw + 1) for rh, rw in RATIOS]  # [121,120,120]
OFFSETS = np.concatenate([[0], np.cumsum(WINDOW_NUMS)]).astype(int)  # [0,121,241,361]
NWIN = int(OFFSETS[-1])     # 361
NPROP = sum(N_LIST)         # 7
OUTW = NPROP * 2 + NWIN     # 375
BIGI = 1.0e6
BIGS = 1.0e30
WMAX = max(WINDOW_NUMS)     # 121


def _split_multi_waits(nc, max_keep=1):
    """walrus in this toolchain rejects instructions carrying more than one
    semaphore wait (e.g. the TileContext tail drain). Hoist extra waits onto
    EventSemaphore instructions inserted just before, on the same engine
    (engines execute in order, so semantics are unchanged)."""
    n = 0
    for f in nc.m.functions:
        for bb in f.blocks:
            out = []
            changed = False
            for ins in bb.instructions:
                si = ins.sync_info
                waits = list(si.on_wait) if (si is not None and si.on_wait) else []
                if len(waits) > max_keep:
                    changed = True
                    for w in waits[:-max_keep]:
                        n += 1
                        es = mybir.InstEventSemaphore(
                            name=f"WSPLIT-{n}", ins=[], outs=[],
                            sync_info=mybir.SyncInfo(on_wait=[w], on_update=[]))
                        es.engine = ins.engine
                        out.append(es)
                    ins.sync_info = mybir.SyncInfo(
                        on_wait=waits[-max_keep:],
                        on_update=list(si.on_update) if si.on_update else [])
                out.append(ins)
            if changed:
                bb.instructions = out
    return n


def _build_module():
    nc = bass.Bass()
    x_ext = nc.declare_dram_parameter("x", [SPC, FLAT], F32, isOutput=False)
    sel_ext = nc.declare_dram_parameter("sel", [128, SPC * SPC], F32, isOutput=False)
    id8_ext = nc.declare_dram_parameter("id8", [SPC, SPC], F32, isOutput=False)
    sm_ext = nc.declare_dram_parameter("smask", [WMAX, NWIN], F32, isOutput=False)
    io_ext = nc.declare_dram_parameter("iotamb", [SPC, NWIN], F32, isOutput=False)
    out_ext = nc.declare_dram_parameter("out", [SPC, OUTW], F32, isOutput=True)

    with TileContext(nc) as tc:
        with tc.tile_pool(name="const", bufs=1) as cpool, \
             tc.tile_pool(name="xs", bufs=4) as xpool, \
             tc.tile_pool(name="work", bufs=2) as wpool, \
             tc.tile_pool(name="pss", bufs=2, space="PSUM") as pss, \
             tc.tile_pool(name="pst", bufs=2, space="PSUM") as pstp, \
             tc.tile_pool(name="psg", bufs=2, space="PSUM") as psgp:

            sel_t = cpool.tile([128, SPC * SPC], F32)
            id8_t = cpool.tile([SPC, SPC], F32)
            sm_t = cpool.tile([WMAX, NWIN], F32)
            io_t = cpool.tile([SPC, NWIN], F32)

            s_all = cpool.tile([SPC, POS], F32)
            out_sb = cpool.tile([SPC, OUTW], F32)

            # ---- channel reduction; selector columns accumulate sample s
            # ---- into PSUM row s (other rows get exact +0.0). DVE folds
            # ---- 16->4 channel groups; PE absorbs the last 4x via two
            # ---- accumulating fp32 matmuls per sample.
            ps = pss.tile([SPC, POS], F32)
            HPF = PF // 2
            QPF = PF // 4
            for s in range(SPC):
                xs = x_ext[s].rearrange("(p f) -> p f", p=128)
                last = s == SPC - 1
                # the last sample streams as [half, quarter, quarter] so the
                # final serial fold chain after the stream ends is shortest
                pieces = ((0, HPF), (HPF, HPF + QPF), (HPF + QPF, PF)) \
                    if last else ((0, PF),)
                for hh, (p0, p1) in enumerate(pieces):
                    xt = xpool.tile([128, PF], F32)
                    w = p1 - p0
                    nc.sync.dma_start(out=xt[:, 0:w], in_=xs[:, p0:p1])
                    if s == 0:
                        for t, csrc in ((sel_t, sel_ext), (id8_t, id8_ext),
                                        (sm_t, sm_ext), (io_t, io_ext)):
                            nc.scalar.dma_start(out=t[:], in_=csrc[:])
                    while w > POS if last else w > 2 * POS:
                        w //= 2
                        nc.vector.tensor_add(xt[:, 0:w], xt[:, 0:w],
                                             xt[:, w:2 * w])
                    for c in range(1 if last else 2):
                        nc.tensor.matmul(ps[:],
                                         lhsT=sel_t[:, SPC * s:SPC * (s + 1)],
                                         rhs=xt[:, POS * c:POS * (c + 1)],
                                         start=(s == 0 and hh == 0 and c == 0),
                                         stop=(last and hh == len(pieces) - 1))

            # ---- pooling: shared unscaled rowsums, scale folded into the
            # ---- column pass. s_all copied from PSUM once (DVE reads of
            # ---- two PSUM operands in one op are not allowed).
            nc.vector.tensor_copy(s_all[:], ps[:])
            sv = s_all[:].rearrange("p (i j) -> p i j", i=FM)
            r3t = wpool.tile([SPC, FM * 12], F32, tag="r3")
            r3 = r3t[:].rearrange("p (i j) -> p i j", i=FM)
            nc.vector.tensor_add(r3[:, :, :], sv[:, :, 0:12], sv[:, :, 1:13])
            nc.vector.tensor_add(r3[:, :, :], r3[:, :, :], sv[:, :, 2:14])
            r4t = wpool.tile([SPC, FM * 11], F32, tag="r4")
            r4 = r4t[:].rearrange("p (i j) -> p i j", i=FM)
            nc.vector.tensor_add(r4[:, :, :], r3[:, :, 0:11], sv[:, :, 3:14])
            r5t = wpool.tile([SPC, FM * 10], F32, tag="r5")
            r5 = r5t[:].rearrange("p (i j) -> p i j", i=FM)
            nc.vector.tensor_add(r5[:, :, :], r4[:, :, 0:10], sv[:, :, 4:14])
            rows = {4: r4, 5: r5, 3: r3}
            for g, (rh, rw) in enumerate(RATIOS):
                st, en = int(OFFSETS[g]), int(OFFSETS[g + 1])
                jw, ih = FM - rw + 1, FM - rh + 1
                scale = 1.0 / (rh * rw)
                rp = rows[rw]
                dst = out_sb[:, 2 * NPROP + st:2 * NPROP + en].rearrange(
                    "p (i j) -> p i j", i=ih)
                if rh in (4, 5):
                    # tree: u[i] = rp[i] + rp[i+1]; dst = u[i] + u[i+2] (+ rp[i+4])
                    ut = wpool.tile([SPC, (FM - 1) * jw], F32, tag=f"u{g}")
                    u = ut[:].rearrange("p (i j) -> p i j", i=FM - 1)
                    nc.vector.tensor_add(u[:, :, :], rp[:, 0:FM - 1, :],
                                         rp[:, 1:FM, :])
                    nc.vector.tensor_add(dst[:, :, :], u[:, 0:ih, :],
                                         u[:, 2:2 + ih, :])
                    if rh == 5:
                        nc.vector.tensor_add(dst[:, :, :], dst[:, :, :],
                                             rp[:, 4:4 + ih, :])
                    nc.vector.tensor_scalar(dst[:, :, :], dst[:, :, :],
                                            scale, None, ALU.mult)
                else:
                    nc.vector.tensor_scalar(dst[:, :, :], rp[:, 0:ih, :],
                                            scale, None, ALU.mult)
                    for hd in range(1, rh):
                        nc.vector.scalar_tensor_tensor(
                            out=dst[:, :, :], in0=rp[:, hd:hd + ih, :],
                            scalar=scale, in1=dst[:, :, :],
                            op0=ALU.mult, op1=ALU.add)

            # ---- greedy NMS per group, samples vectorized on partitions.
            # Index extraction: scr = (masked == max) * iota2 where
            # iota2[j] = (NWIN - j)/512 > 0; reduce_max(scr) encodes the
            # first-argmax index (host decodes idx = NWIN - 512*v).
            # Suppression row gather: scrT.T @ smask = v * S[idx, :] with
            # v > 0, so masked += -1e30 * supp kills suppressed windows.
            col = 0
            for g, (n_g, wg) in enumerate(zip(N_LIST, WINDOW_NUMS)):
                st, en = int(OFFSETS[g]), int(OFFSETS[g + 1])
                masked = wpool.tile([SPC, wg], F32, tag=f"masked{g}")
                for k in range(n_g):
                    src_ap = (out_sb[:, 2 * NPROP + st:2 * NPROP + en]
                              if k == 0 else masked[:])
                    sc_col = out_sb[:, NPROP + col:NPROP + col + 1]
                    nc.vector.tensor_reduce(sc_col, src_ap, axis=AXX, op=ALU.max)
                    scr = wpool.tile([SPC, wg], F32, tag=f"scr{g}")
                    nc.vector.scalar_tensor_tensor(
                        out=scr[:], in0=src_ap, scalar=sc_col,
                        in1=io_t[:, st:en], op0=ALU.is_equal, op1=ALU.mult,
                        accum_out=out_sb[:, col:col + 1])
                    if k < n_g - 1:
                        pT = pstp.tile([WMAX, SPC], F32, tag="pT")
                        nc.tensor.transpose(pT[0:wg, :], scr[:], id8_t[:])
                        ohT = wpool.tile([WMAX, SPC], F32, tag=f"ohT{g}")
                        nc.vector.tensor_copy(ohT[0:wg, :], pT[0:wg, :])
                        pS = psgp.tile([SPC, WMAX], F32, tag="pS")
                        nc.tensor.matmul(pS[:, 0:wg], lhsT=ohT[0:wg, :],
                                         rhs=sm_t[0:wg, st:en],
                                         start=True, stop=True)
                        nc.vector.scalar_tensor_tensor(
                            out=masked[:], in0=pS[:, 0:wg], scalar=-BIGS,
                            in1=src_ap, op0=ALU.mult, op1=ALU.add)
                    col += 1

            nc.sync.dma_start(out=out_ext[:, 2 * NPROP:],
                              in_=out_sb[:, 2 * NPROP:])
            nc.sync.dma_start(out=out_ext[:, 0:2 * NPROP],
                              in_=out_sb[:, 0:2 * NPROP])

    _split_multi_waits(nc)
    return nc


def _host_constants(coordinates):
    co = np.asarray(coordinates).astype(np.int64)  # [361, 4]
    sel = np.zeros((128, SPC * SPC), np.float32)
    for s in range(SPC):
        sel[:, SPC * s + s] = 1.0
    id8 = np.eye(SPC, dtype=np.float32)
    smask = np.zeros((WMAX, NWIN), np.float32)
    for g in range(len(RATIOS)):
        st, en = int(OFFSETS[g]), int(OFFSETS[g + 1])
        b = co[st:en]
        x0 = np.maximum(b[:, None, 0], b[None, :, 0])
        y0 = np.maximum(b[:, None, 1], b[None, :, 1])
        x1 = np.minimum(b[:, None, 2], b[None, :, 2])
        y1 = np.minimum(b[:, None, 3], b[None, :, 3])
        inter = np.maximum(x1 - x0, 0) * np.maximum(y1 - y0, 0)
        area = (b[:, 2] - b[:, 0]) * (b[:, 3] - b[:, 1])
        union = area[:, None] + area[None, :] - inter
        # iou >= 0.25 exactly, in integer arithmetic
        sup = (4 * inter >= union).astype(np.float32)
        smask[0:en - st, st:en] = sup
    iotamb = ((NWIN - np.arange(NWIN, dtype=np.float64)) / 512.0).astype(np.float32)
    iotamb = np.broadcast_to(iotamb, (SPC, NWIN)).copy()
    return sel, id8, smask, iotamb


_NC_CACHE = {}


def _get_module():
    if "nc" not in _NC_CACHE:
        _NC_CACHE["nc"] = _build_module()
    return _NC_CACHE["nc"]


def run(inputs, trace=False):
    x = np.asarray(inputs["x"], dtype=np.float32)
    coordinates = np.asarray(inputs["coordinates"])
    assert x.shape == (B, C, FM, FM), x.shape
    sel, id8, smask, iotamb = _host_constants(coordinates)

    nc = _get_module()
    in_maps = []
    for i in range(N_CORES):
        shard = np.ascontiguousarray(
            x[i * SPC:(i + 1) * SPC].reshape(SPC, FLAT))
        in_maps.append({
            "x": shard, "sel": sel, "id8": id8,
            "smask": smask, "iotamb": iotamb,
        })
    res = run_bass_kernel_spmd(nc, in_maps, core_ids=list(range(N_CORES)),
                               trace=trace)
    outs = [res.results[i]["out"] for i in range(N_CORES)]
    full = np.concatenate(outs, axis=0)  # [64, 375]
    idx = np.rint(NWIN - 512.0 * full[:, 0:NPROP].astype(np.float64)).astype(np.int32)
    prop_scores = full[:, NPROP:2 * NPROP].astype(np.float32)
    all_scores = full[:, 2 * NPROP:].astype(np.float32)
    return (idx, prop_scores, all_scores), res


def kernel(**inputs):
    (idx, prop_scores, all_scores), _ = run(inputs, trace=False)
    return idx, prop_scores, all_scores
